# revision 6
# baseline (speedup 1.0000x reference)
"""Trainium2 Bass kernel for CrossAttention (B=2, T=S=2048, E=1024, H=16, D=64).

Sharding: 8 cores = 2 (batch) x 4 (head groups of 4 heads).
Each core computes, for its (b, g):
  - Q/K projections in feature-major layout: QT/KT = [256, 2048]
  - V projection in sequence-major layout with an appended ones column per
    head (gives the softmax denominator for free from the attn@V matmul)
  - causal flash-style attention (additive -1e30 mask folded into PSUM via an
    extra matmul with an identity lhsT; exp on ScalarE with the 1/sqrt(d)
    scale and per-partition key-padding bias folded in)
  - output projection partial: outT_partial = Wo[:, group].T-style [1024, 2048]
Host: shards/transposes inputs, gathers partials, sums over the 4 groups per
batch and adds bo.

All device transposes are avoided by preparing feature-major inputs host-side.
"""

import numpy as np

import concourse.bass as bass
import concourse.bacc as bacc
import concourse.mybir as mybir
import concourse.tile as tile
from concourse.bass_utils import run_bass_kernel_spmd

P = 128
T = 2048          # target length
S = 2048          # source length
E = 1024          # embed dim
D = 64            # head dim
GC = 256          # channels per group (4 heads * 64)
NHL = 4           # heads per core (local)
KB = E // P       # 8 full k-blocks for the E contraction
TJ = 512          # t-chunk width
NTJ = T // TJ     # 4
NSB = S // P      # 16 s-blocks
VC = NHL * (D + 1)  # 260 = V-projection cols (64 V + 1 ones per head)
SCALE = float(D) ** -0.5  # 0.125
NEG = -1.0e30

F32 = mybir.dt.float32
BF16 = mybir.dt.bfloat16

# dtype configuration knobs (f32 first for correctness; can be relaxed later)
DT_PROJ = F32     # Q/K/V projection matmul operand dtype (x tiles + weights)
DT_QK = F32       # QT/KT storage dtype -> scores matmul dtype
DT_EXP = F32      # exp output / V storage dtype -> attn@V matmul dtype
DT_AO = F32       # normalized attention-out dtype -> o-proj matmul dtype


def _build_program():
    nc = bacc.Bacc()

    xq = nc.dram_tensor("xq_t", [E, T], F32, kind="ExternalInput")
    xk = nc.dram_tensor("xk_t", [E, S], F32, kind="ExternalInput")
    xv = nc.dram_tensor("xv_t", [E, S], F32, kind="ExternalInput")
    wq = nc.dram_tensor("wq_t", [E + 1, GC], F32, kind="ExternalInput")
    wk = nc.dram_tensor("wk_t", [E + 1, GC], F32, kind="ExternalInput")
    wv = nc.dram_tensor("wv_t", [E + 1, VC], F32, kind="ExternalInput")
    wo = nc.dram_tensor("wo_t", [GC, E], F32, kind="ExternalInput")
    # single consolidated constant tensor -> one DMA -> one wait semaphore
    # cols: [0:128] identity, [128:128+896] causal mask, [last 16] pad bias
    CW = P + 384 + TJ + NSB
    cst = nc.dram_tensor("cst", [P, CW], F32, kind="ExternalInput")
    out_t = nc.dram_tensor("out_t", [E, T], F32, kind="ExternalOutput")

    with tile.TileContext(nc) as tc:
        with (
            tc.tile_pool(name="consts", bufs=1) as cpool,
            tc.tile_pool(name="xs", bufs=10) as xpool,
            tc.tile_pool(name="persist", bufs=1) as ppool,
            tc.tile_pool(name="expw", bufs=3) as epool,
            tc.tile_pool(name="ao", bufs=1) as apool,
            tc.tile_pool(name="aon", bufs=2) as npool,
            tc.tile_pool(name="ft", bufs=2) as fpool,
            tc.tile_pool(name="ps", bufs=1, space="PSUM") as pspool,
        ):
            # ---- constants / weights to SBUF ----
            wq_sb = cpool.tile([P, KB + 1, GC], DT_PROJ, name="wq_sb")
            wk_sb = cpool.tile([P, KB + 1, GC], DT_PROJ, name="wk_sb")
            wv_sb = cpool.tile([P, KB + 1, VC], DT_PROJ, name="wv_sb")
            wo_sb = cpool.tile([P, 2, E], DT_AO, name="wo_sb")
            cst_sb = cpool.tile([P, CW], F32, name="cst_sb")
            ident_sb = cst_sb[:, 0:P]
            mneg_sb = cst_sb[:, P : P + 384 + TJ]
            padb_sb = cst_sb[:, P + 384 + TJ :]
            ones_sb = cpool.tile([1, TJ], DT_PROJ, name="ones_sb")

            for w_sb, w_dram in ((wq_sb, wq), (wk_sb, wk), (wv_sb, wv)):
                ncols = w_sb.shape[2]
                nc.sync.dma_start(
                    w_sb[:, :KB, :],
                    w_dram[: KB * P, :].rearrange("(kb p) c -> p kb c", p=P),
                )
                nc.sync.dma_start(w_sb[0:1, KB, :], w_dram[KB * P : KB * P + 1, :])
            nc.sync.dma_start(
                wo_sb[:], wo.rearrange("(cc p) o -> p cc o", p=P)
            )
            nc.sync.dma_start(cst_sb[:], cst[:])
            nc.any.memset(ones_sb[:], 1.0)

            # ---- persistent activations ----
            qt_sb = ppool.tile([P, 2, T], DT_QK, name="qt_sb")
            kt_sb = ppool.tile([P, 2, S], DT_QK, name="kt_sb")
            v_sb = ppool.tile([P, NSB, VC], DT_EXP, name="v_sb")
            aoTn = ppool.tile([P, 2, T], DT_AO, name="aoTn")

            # ---- Q / K projections (channel-major output) ----
            for x_dram, w_sb, dst in ((xq, wq_sb, qt_sb), (xk, wk_sb, kt_sb)):
                for j in range(NTJ):
                    xt = []
                    for kb in range(KB):
                        t_ = xpool.tile([P, TJ], DT_PROJ, tag="xs", name="xt")
                        nc.sync.dma_start(
                            t_[:],
                            x_dram[kb * P : (kb + 1) * P, j * TJ : (j + 1) * TJ],
                        )
                        xt.append(t_)
                    for mc in range(2):
                        ps = pspool.tile([P, TJ], F32, tag="ps_pr", name="ps_pr")
                        for kb in range(KB):
                            nc.tensor.matmul(
                                ps[:],
                                lhsT=w_sb[:, kb, mc * P : (mc + 1) * P],
                                rhs=xt[kb][:],
                                start=(kb == 0),
                                stop=False,
                            )
                        nc.tensor.matmul(
                            ps[:],
                            lhsT=w_sb[0:1, KB, mc * P : (mc + 1) * P],
                            rhs=ones_sb[0:1, :],
                            start=False,
                            stop=True,
                        )
                        nc.any.tensor_copy(
                            out=dst[:, mc, j * TJ : (j + 1) * TJ], in_=ps[:]
                        )

            # ---- V projection (sequence-major, 65 cols per head) ----
            for sj in range(NTJ):
                xt = []
                for kb in range(KB):
                    t_ = xpool.tile([P, TJ], DT_PROJ, tag="xs", name="xvt")
                    nc.sync.dma_start(
                        t_[:], xv[kb * P : (kb + 1) * P, sj * TJ : (sj + 1) * TJ]
                    )
                    xt.append(t_)
                for ii in range(TJ // P):
                    i = sj * (TJ // P) + ii
                    ps = pspool.tile([P, TJ], F32, tag="ps_pr", name="ps_v")
                    for kb in range(KB):
                        nc.tensor.matmul(
                            ps[:, :VC],
                            lhsT=xt[kb][:, ii * P : (ii + 1) * P],
                            rhs=wv_sb[:, kb, :],
                            start=(kb == 0),
                            stop=False,
                        )
                    nc.tensor.matmul(
                        ps[:, :VC],
                        lhsT=ones_sb[0:1, 0:P],
                        rhs=wv_sb[0:1, KB, :],
                        start=False,
                        stop=True,
                    )
                    nc.any.tensor_copy(out=v_sb[:, i, :], in_=ps[:, :VC])

            # ---- attention, head pairs (lh=0 at partitions 0-63, lh=1 at 64-127)
            for hp in range(2):
                aoTS = [
                    apool.tile([D + 1, T], F32, tag=f"aoTS_{lh}", name="aoTS")
                    for lh in range(2)
                ]
                for j in range(NTJ):
                    nsb_j = 4 * j + 4  # s-blocks 0..4j+3 (causal)
                    av_ps = [
                        pspool.tile([P, TJ], F32, tag=f"ps_av{lh}", name="ps_av")
                        for lh in range(2)
                    ]
                    for i in range(nsb_j):
                        r = i - 4 * j
                        for lh in range(2):
                            base = D * lh
                            ps = pspool.tile([P, TJ], F32, tag="ps_sc", name="ps_sc")
                            if r >= 0:
                                off = 384 - P * r
                                nc.tensor.matmul(
                                    ps[:],
                                    lhsT=ident_sb,
                                    rhs=mneg_sb[:, off : off + TJ],
                                    start=True,
                                    stop=False,
                                )
                            nc.tensor.matmul(
                                ps[:],
                                lhsT=kt_sb[base : base + D, hp, i * P : (i + 1) * P],
                                rhs=qt_sb[base : base + D, hp, j * TJ : (j + 1) * TJ],
                                start=(r < 0),
                                stop=True,
                            )
                            et = epool.tile([P, TJ], DT_EXP, tag="exp", name="et")
                            nc.scalar.activation(
                                et[:],
                                ps[:],
                                mybir.ActivationFunctionType.Exp,
                                bias=padb_sb[:, i : i + 1],
                                scale=SCALE,
                            )
                            h65 = (hp * 2 + lh) * (D + 1)
                            nc.tensor.matmul(
                                av_ps[lh][: D + 1, :],
                                lhsT=v_sb[:, i, h65 : h65 + D + 1],
                                rhs=et[:],
                                start=(i == 0),
                                stop=(i == nsb_j - 1),
                            )
                    for lh in range(2):
                        nc.any.tensor_copy(
                            out=aoTS[lh][:, j * TJ : (j + 1) * TJ],
                            in_=av_ps[lh][: D + 1, :],
                        )
                # normalize the two heads of this pair
                for lh in range(2):
                    rrow = apool.tile([1, T], F32, tag="rrow", name="rrow")
                    nc.sync.dma_start(rrow[:], aoTS[lh][D : D + 1, :])
                    rcp = apool.tile([1, T], F32, tag="rcp", name="rcp")
                    nc.vector.reciprocal(rcp[:], rrow[:])
                    if lh == 0:
                        dst_slice = aoTn[0:D, hp, :]
                        tmp = None
                    else:
                        tmp = npool.tile([D, T], DT_AO, tag="aon", name="aon")
                        dst_slice = tmp[:]
                    for jj in range(NTJ):
                        rb = pspool.tile([P, TJ], F32, tag="ps_sc", name="ps_rb")
                        nc.tensor.matmul(
                            rb[:D, :],
                            lhsT=ones_sb[0:1, 0:D],
                            rhs=rcp[0:1, jj * TJ : (jj + 1) * TJ],
                            start=True,
                            stop=True,
                        )
                        nc.vector.tensor_mul(
                            out=dst_slice[:, jj * TJ : (jj + 1) * TJ],
                            in0=aoTS[lh][0:D, jj * TJ : (jj + 1) * TJ],
                            in1=rb[:D, :],
                        )
                    if lh == 1:
                        # partition shift 0-63 -> 64-127 via SBUF-to-SBUF DMA
                        nc.sync.dma_start(aoTn[D : 2 * D, hp, :], tmp[:])

            # ---- output projection (partial, feature-major) ----
            for mc in range(KB):
                ft = fpool.tile([P, T], F32, tag="ft", name="ft")
                for j in range(NTJ):
                    ps = pspool.tile([P, TJ], F32, tag="ps_pr", name="ps_o")
                    for cc in range(2):
                        nc.tensor.matmul(
                            ps[:],
                            lhsT=wo_sb[:, cc, mc * P : (mc + 1) * P],
                            rhs=aoTn[:, cc, j * TJ : (j + 1) * TJ],
                            start=(cc == 0),
                            stop=(cc == 1),
                        )
                    nc.any.tensor_copy(out=ft[:, j * TJ : (j + 1) * TJ], in_=ps[:])
                nc.sync.dma_start(out_t[mc * P : (mc + 1) * P, :], ft[:])

    nc.compile()
    return nc


_NC_CACHE = None


def _get_nc():
    global _NC_CACHE
    if _NC_CACHE is None:
        _NC_CACHE = _build_program()
    return _NC_CACHE


def _make_in_maps(query, key, value, key_padding_mask, Wq, bq, Wk, bk, Wv, bv, Wo, bo):
    f32 = np.float32
    query = np.asarray(query, f32)
    key = np.asarray(key, f32)
    value = np.asarray(value, f32)
    kpm = np.asarray(key_padding_mask, bool)
    Wq, bq = np.asarray(Wq, f32), np.asarray(bq, f32)
    Wk, bk = np.asarray(Wk, f32), np.asarray(bk, f32)
    Wv, bv = np.asarray(Wv, f32), np.asarray(bv, f32)
    Wo = np.asarray(Wo, f32)

    # constants shared by all cores
    su = np.arange(384 + TJ)[None, :] < (np.arange(P)[:, None] + 384)
    mneg_np = np.where(su, NEG, 0.0).astype(f32)
    ident_np = np.eye(P, dtype=f32)

    in_maps = []
    for c in range(8):
        b, g = divmod(c, 4)
        cols = slice(g * GC, (g + 1) * GC)

        wq_t = np.concatenate([Wq[cols, :].T, bq[cols][None, :]], axis=0)
        wk_t = np.concatenate([Wk[cols, :].T, bk[cols][None, :]], axis=0)

        wv_t = np.zeros((E + 1, VC), f32)
        for h in range(NHL):
            ch = slice(g * GC + h * D, g * GC + (h + 1) * D)
            wv_t[:E, h * (D + 1) : h * (D + 1) + D] = Wv[ch, :].T
            wv_t[E, h * (D + 1) : h * (D + 1) + D] = bv[ch]
            wv_t[E, h * (D + 1) + D] = 1.0  # ones column -> softmax denominator

        wo_t = np.ascontiguousarray(Wo[:, cols].T)

        padb_np = np.where(kpm[b], NEG, 0.0).astype(f32).reshape(NSB, P).T
        cst_np = np.ascontiguousarray(
            np.concatenate([ident_np, mneg_np, padb_np], axis=1)
        )

        in_maps.append(
            {
                "xq_t": np.ascontiguousarray(query[b].T),
                "xk_t": np.ascontiguousarray(key[b].T),
                "xv_t": np.ascontiguousarray(value[b].T),
                "wq_t": np.ascontiguousarray(wq_t),
                "wk_t": np.ascontiguousarray(wk_t),
                "wv_t": wv_t,
                "wo_t": wo_t,
                "cst": cst_np,
            }
        )
    return in_maps


def kernel(**inputs) -> np.ndarray:
    nc = _get_nc()
    in_maps = _make_in_maps(**inputs)
    res = run_bass_kernel_spmd(nc, in_maps, core_ids=list(range(8)))
    bo = np.asarray(inputs["bo"], np.float32)
    B = inputs["query"].shape[0]
    out = np.zeros((B, T, E), np.float32)
    for c in range(8):
        b = c // 4
        out[b] += res.results[c]["out_t"].T
    out += bo[None, None, :]
    return out


# revision 11
# speedup vs baseline: 1.7321x; 1.7321x over previous
"""Trainium2 Bass kernel for CrossAttention (B=2, T=S=2048, E=1024, H=16, D=64).

Sharding: 8 cores = 2 (batch) x 4 (head groups of 4 heads).
Each core computes, for its (b, g):
  - Q/K projections in feature-major layout: QT/KT = [256, 2048]
  - V projection in sequence-major layout with an appended ones column per
    head (gives the softmax denominator for free from the attn@V matmul)
  - causal flash-style attention (additive -1e30 mask folded into PSUM via an
    extra matmul with an identity lhsT; exp on ScalarE with the 1/sqrt(d)
    scale and per-partition key-padding bias folded in)
  - output projection partial: outT_partial = Wo[:, group].T-style [1024, 2048]
Host: shards/transposes inputs, gathers partials, sums over the 4 groups per
batch and adds bo.

All device transposes are avoided by preparing feature-major inputs host-side.
"""

import ml_dtypes
import numpy as np

import concourse.bass as bass
import concourse.bacc as bacc
import concourse.mybir as mybir
import concourse.tile as tile
from concourse.bass_utils import run_bass_kernel_spmd

P = 128
T = 2048          # target length
S = 2048          # source length
E = 1024          # embed dim
D = 64            # head dim
GC = 256          # channels per group (4 heads * 64)
NHL = 4           # heads per core (local)
KB = E // P       # 8 full k-blocks for the E contraction
TJ = 512          # t-chunk width
NTJ = T // TJ     # 4
NSB = S // P      # 16 s-blocks
VC = NHL * (D + 1)  # 260 = V-projection cols (64 V + 1 ones per head)
SCALE = float(D) ** -0.5  # 0.125
NEG = -1.0e30

F32 = mybir.dt.float32
BF16 = mybir.dt.bfloat16
F16 = mybir.dt.float16

# dtype knobs: fp16 runs the PE at 1 cyc/row (vs fp32's 2 half-speed passes)
# with an 11-bit mantissa; all tensors here are O(1)-scaled so range is safe.
DT_PROJ = F16     # Q/K/V projection matmul operand dtype (x tiles + weights)
DT_QK = F16       # QT/KT storage dtype -> scores matmul dtype
DT_EXP = F16      # exp output / V storage dtype -> attn@V matmul dtype
DT_AO = F16       # normalized attention-out dtype -> o-proj matmul dtype
NEG_H = -60000.0  # causal-mask additive constant (fits fp16; exp(scale*x)->0)


def _build_program():
    nc = bacc.Bacc()

    xq = nc.dram_tensor("xq_t", [E, T], DT_PROJ, kind="ExternalInput")
    xk = nc.dram_tensor("xk_t", [E, S], DT_PROJ, kind="ExternalInput")
    xv = nc.dram_tensor("xv_t", [E, S], DT_PROJ, kind="ExternalInput")
    wq = nc.dram_tensor("wq_t", [E + 1, GC], DT_PROJ, kind="ExternalInput")
    wk = nc.dram_tensor("wk_t", [E + 1, GC], DT_PROJ, kind="ExternalInput")
    wv = nc.dram_tensor("wv_t", [E + 1, VC], DT_PROJ, kind="ExternalInput")
    wo = nc.dram_tensor("wo_t", [GC, E], DT_AO, kind="ExternalInput")
    # consolidated constants: identity + causal mask in fp16 (one DMA so the
    # mask matmul waits on a single queue), key-padding bias separately in f32
    CW = P + 384 + TJ
    cst = nc.dram_tensor("cst", [P, CW], DT_QK, kind="ExternalInput")
    padb = nc.dram_tensor("padb", [P, NSB], F32, kind="ExternalInput")
    out_t = nc.dram_tensor("out_t", [E, T], F32, kind="ExternalOutput")

    with tile.TileContext(nc) as tc:
        with (
            tc.tile_pool(name="consts", bufs=1) as cpool,
            tc.tile_pool(name="xs", bufs=10) as xpool,
            tc.tile_pool(name="persist", bufs=1) as ppool,
            tc.tile_pool(name="expw", bufs=3) as epool,
            tc.tile_pool(name="ao", bufs=1) as apool,
            tc.tile_pool(name="aon", bufs=2) as npool,
            tc.tile_pool(name="ft", bufs=2) as fpool,
            tc.tile_pool(name="ps", bufs=1, space="PSUM") as pspool,
        ):
            # ---- constants / weights to SBUF ----
            wq_sb = cpool.tile([P, KB + 1, GC], DT_PROJ, name="wq_sb")
            wk_sb = cpool.tile([P, KB + 1, GC], DT_PROJ, name="wk_sb")
            wv_sb = cpool.tile([P, KB + 1, VC], DT_PROJ, name="wv_sb")
            wo_sb = cpool.tile([P, 2, E], DT_AO, name="wo_sb")
            cst_sb = cpool.tile([P, CW], DT_QK, name="cst_sb")
            ident_sb = cst_sb[:, 0:P]
            mneg_sb = cst_sb[:, P : P + 384 + TJ]
            padb_sb = cpool.tile([P, NSB], F32, name="padb_sb")
            ones_sb = cpool.tile([1, TJ], DT_PROJ, name="ones_sb")
            ones32_sb = cpool.tile([1, D], F32, name="ones32_sb")

            for w_sb, w_dram in ((wq_sb, wq), (wk_sb, wk), (wv_sb, wv)):
                ncols = w_sb.shape[2]
                nc.sync.dma_start(
                    w_sb[:, :KB, :],
                    w_dram[: KB * P, :].rearrange("(kb p) c -> p kb c", p=P),
                )
                nc.sync.dma_start(w_sb[0:1, KB, :], w_dram[KB * P : KB * P + 1, :])
            nc.sync.dma_start(
                wo_sb[:], wo.rearrange("(cc p) o -> p cc o", p=P)
            )
            nc.sync.dma_start(cst_sb[:], cst[:])
            nc.sync.dma_start(padb_sb[:], padb[:])
            nc.any.memset(ones_sb[:], 1.0)
            nc.any.memset(ones32_sb[:], 1.0)

            # ---- persistent activations ----
            qt_sb = ppool.tile([P, 2, T], DT_QK, name="qt_sb")
            kt_sb = ppool.tile([P, 2, S], DT_QK, name="kt_sb")
            v_sb = ppool.tile([P, NSB, VC], DT_EXP, name="v_sb")
            aoTn = ppool.tile([P, 2, T], DT_AO, name="aoTn")

            # ---- Q / K projections (channel-major output) ----
            for x_dram, w_sb, dst in ((xq, wq_sb, qt_sb), (xk, wk_sb, kt_sb)):
                for j in range(NTJ):
                    xt = []
                    for kb in range(KB):
                        t_ = xpool.tile([P, TJ], DT_PROJ, tag="xs", name="xt")
                        nc.sync.dma_start(
                            t_[:],
                            x_dram[kb * P : (kb + 1) * P, j * TJ : (j + 1) * TJ],
                        )
                        xt.append(t_)
                    for mc in range(2):
                        ps = pspool.tile([P, TJ], F32, tag="ps_pr", name="ps_pr")
                        for kb in range(KB):
                            nc.tensor.matmul(
                                ps[:],
                                lhsT=w_sb[:, kb, mc * P : (mc + 1) * P],
                                rhs=xt[kb][:],
                                start=(kb == 0),
                                stop=False,
                            )
                        nc.tensor.matmul(
                            ps[:],
                            lhsT=w_sb[0:1, KB, mc * P : (mc + 1) * P],
                            rhs=ones_sb[0:1, :],
                            start=False,
                            stop=True,
                        )
                        nc.any.tensor_copy(
                            out=dst[:, mc, j * TJ : (j + 1) * TJ], in_=ps[:]
                        )

            # ---- V projection (sequence-major, 65 cols per head) ----
            for sj in range(NTJ):
                xt = []
                for kb in range(KB):
                    t_ = xpool.tile([P, TJ], DT_PROJ, tag="xs", name="xvt")
                    nc.sync.dma_start(
                        t_[:], xv[kb * P : (kb + 1) * P, sj * TJ : (sj + 1) * TJ]
                    )
                    xt.append(t_)
                for ii in range(TJ // P):
                    i = sj * (TJ // P) + ii
                    ps = pspool.tile([P, TJ], F32, tag="ps_pr", name="ps_v")
                    for kb in range(KB):
                        nc.tensor.matmul(
                            ps[:, :VC],
                            lhsT=xt[kb][:, ii * P : (ii + 1) * P],
                            rhs=wv_sb[:, kb, :],
                            start=(kb == 0),
                            stop=False,
                        )
                    nc.tensor.matmul(
                        ps[:, :VC],
                        lhsT=ones_sb[0:1, 0:P],
                        rhs=wv_sb[0:1, KB, :],
                        start=False,
                        stop=True,
                    )
                    nc.any.tensor_copy(out=v_sb[:, i, :], in_=ps[:, :VC])

            # ---- attention, head pairs (lh=0 at partitions 0-63, lh=1 at 64-127)
            for hp in range(2):
                aoTS = [
                    apool.tile([D + 1, T], F32, tag=f"aoTS_{lh}", name="aoTS")
                    for lh in range(2)
                ]
                for j in range(NTJ):
                    nsb_j = 4 * j + 4  # s-blocks 0..4j+3 (causal)
                    av_ps = [
                        pspool.tile([P, TJ], F32, tag=f"ps_av{lh}", name="ps_av")
                        for lh in range(2)
                    ]
                    for i in range(nsb_j):
                        r = i - 4 * j
                        for lh in range(2):
                            base = D * lh
                            ps = pspool.tile([P, TJ], F32, tag="ps_sc", name="ps_sc")
                            if r >= 0:
                                off = 384 - P * r
                                nc.tensor.matmul(
                                    ps[:],
                                    lhsT=ident_sb,
                                    rhs=mneg_sb[:, off : off + TJ],
                                    start=True,
                                    stop=False,
                                )
                            nc.tensor.matmul(
                                ps[:],
                                lhsT=kt_sb[base : base + D, hp, i * P : (i + 1) * P],
                                rhs=qt_sb[base : base + D, hp, j * TJ : (j + 1) * TJ],
                                start=(r < 0),
                                stop=True,
                            )
                            et = epool.tile([P, TJ], DT_EXP, tag="exp", name="et")
                            nc.scalar.activation(
                                et[:],
                                ps[:],
                                mybir.ActivationFunctionType.Exp,
                                bias=padb_sb[:, i : i + 1],
                                scale=SCALE,
                            )
                            h65 = (hp * 2 + lh) * (D + 1)
                            nc.tensor.matmul(
                                av_ps[lh][: D + 1, :],
                                lhsT=v_sb[:, i, h65 : h65 + D + 1],
                                rhs=et[:],
                                start=(i == 0),
                                stop=(i == nsb_j - 1),
                            )
                    for lh in range(2):
                        nc.any.tensor_copy(
                            out=aoTS[lh][:, j * TJ : (j + 1) * TJ],
                            in_=av_ps[lh][: D + 1, :],
                        )
                # normalize the two heads of this pair
                for lh in range(2):
                    rrow = apool.tile([1, T], F32, tag="rrow", name="rrow")
                    nc.sync.dma_start(rrow[:], aoTS[lh][D : D + 1, :])
                    if lh == 0:
                        dst_slice = aoTn[0:D, hp, :]
                        tmp = None
                    else:
                        tmp = npool.tile([D, T], DT_AO, tag="aon", name="aon")
                        dst_slice = tmp[:]
                    for jj in range(NTJ):
                        rb = pspool.tile([P, TJ], F32, tag="ps_sc", name="ps_rb")
                        nc.tensor.matmul(
                            rb[:D, :],
                            lhsT=ones32_sb[0:1, 0:D],
                            rhs=rrow[0:1, jj * TJ : (jj + 1) * TJ],
                            start=True,
                            stop=True,
                        )
                        rbc = npool.tile([D, TJ], F32, tag="rbc", name="rbc", bufs=3)
                        nc.vector.reciprocal_approx_fast(rbc[:], rb[:D, :])
                        nc.vector.tensor_mul(
                            out=dst_slice[:, jj * TJ : (jj + 1) * TJ],
                            in0=aoTS[lh][0:D, jj * TJ : (jj + 1) * TJ],
                            in1=rbc[:],
                        )
                    if lh == 1:
                        # partition shift 0-63 -> 64-127 via SBUF-to-SBUF DMA
                        nc.sync.dma_start(aoTn[D : 2 * D, hp, :], tmp[:])

            # ---- output projection (partial, feature-major) ----
            for mc in range(KB):
                ft = fpool.tile([P, T], F32, tag="ft", name="ft")
                for j in range(NTJ):
                    ps = pspool.tile([P, TJ], F32, tag="ps_pr", name="ps_o")
                    for cc in range(2):
                        nc.tensor.matmul(
                            ps[:],
                            lhsT=wo_sb[:, cc, mc * P : (mc + 1) * P],
                            rhs=aoTn[:, cc, j * TJ : (j + 1) * TJ],
                            start=(cc == 0),
                            stop=(cc == 1),
                        )
                    nc.any.tensor_copy(out=ft[:, j * TJ : (j + 1) * TJ], in_=ps[:])
                nc.sync.dma_start(out_t[mc * P : (mc + 1) * P, :], ft[:])

    nc.compile()
    return nc


_NC_CACHE = None


def _get_nc():
    global _NC_CACHE
    if _NC_CACHE is None:
        _NC_CACHE = _build_program()
    return _NC_CACHE


def _make_in_maps(query, key, value, key_padding_mask, Wq, bq, Wk, bk, Wv, bv, Wo, bo):
    f32 = np.float32
    query = np.asarray(query, f32)
    key = np.asarray(key, f32)
    value = np.asarray(value, f32)
    kpm = np.asarray(key_padding_mask, bool)
    Wq, bq = np.asarray(Wq, f32), np.asarray(bq, f32)
    Wk, bk = np.asarray(Wk, f32), np.asarray(bk, f32)
    Wv, bv = np.asarray(Wv, f32), np.asarray(bv, f32)
    Wo = np.asarray(Wo, f32)

    # constants shared by all cores
    f16 = np.float16
    su = np.arange(384 + TJ)[None, :] < (np.arange(P)[:, None] + 384)
    mneg_np = np.where(su, NEG_H, 0.0).astype(f16)
    ident_np = np.eye(P, dtype=f16)

    in_maps = []
    for c in range(8):
        b, g = divmod(c, 4)
        cols = slice(g * GC, (g + 1) * GC)

        wq_t = np.concatenate([Wq[cols, :].T, bq[cols][None, :]], axis=0).astype(f16)
        wk_t = np.concatenate([Wk[cols, :].T, bk[cols][None, :]], axis=0).astype(f16)

        wv_t = np.zeros((E + 1, VC), f16)
        for h in range(NHL):
            ch = slice(g * GC + h * D, g * GC + (h + 1) * D)
            wv_t[:E, h * (D + 1) : h * (D + 1) + D] = Wv[ch, :].T
            wv_t[E, h * (D + 1) : h * (D + 1) + D] = bv[ch]
            wv_t[E, h * (D + 1) + D] = 1.0  # ones column -> softmax denominator

        wo_t = np.ascontiguousarray(Wo[:, cols].T.astype(f16))

        padb_np = np.ascontiguousarray(
            np.where(kpm[b], NEG, 0.0).astype(f32).reshape(NSB, P).T
        )
        cst_np = np.ascontiguousarray(np.concatenate([ident_np, mneg_np], axis=1))

        in_maps.append(
            {
                "xq_t": np.ascontiguousarray(query[b].T.astype(f16)),
                "xk_t": np.ascontiguousarray(key[b].T.astype(f16)),
                "xv_t": np.ascontiguousarray(value[b].T.astype(f16)),
                "wq_t": np.ascontiguousarray(wq_t),
                "wk_t": np.ascontiguousarray(wk_t),
                "wv_t": wv_t,
                "wo_t": wo_t,
                "cst": cst_np,
                "padb": padb_np,
            }
        )
    return in_maps


def kernel(**inputs) -> np.ndarray:
    nc = _get_nc()
    in_maps = _make_in_maps(**inputs)
    res = run_bass_kernel_spmd(nc, in_maps, core_ids=list(range(8)))
    bo = np.asarray(inputs["bo"], np.float32)
    B = inputs["query"].shape[0]
    out = np.zeros((B, T, E), np.float32)
    for c in range(8):
        b = c // 4
        out[b] += res.results[c]["out_t"].T
    out += bo[None, None, :]
    return out


# revision 14
# speedup vs baseline: 2.2063x; 1.2737x over previous
"""Trainium2 Bass kernel for CrossAttention (B=2, T=S=2048, E=1024, H=16, D=64).

Sharding: 8 cores = 2 (batch) x 4 (head groups of 4 heads).
Each core computes, for its (b, g):
  - Q/K projections in feature-major layout: QT/KT = [256, 2048]
  - V projection in sequence-major layout with an appended ones column per
    head (gives the softmax denominator for free from the attn@V matmul)
  - causal flash-style attention (additive -1e30 mask folded into PSUM via an
    extra matmul with an identity lhsT; exp on ScalarE with the 1/sqrt(d)
    scale and per-partition key-padding bias folded in)
  - output projection partial: outT_partial = Wo[:, group].T-style [1024, 2048]
Host: shards/transposes inputs, gathers partials, sums over the 4 groups per
batch and adds bo.

All device transposes are avoided by preparing feature-major inputs host-side.
"""

import ml_dtypes
import numpy as np

import concourse.bass as bass
import concourse.bacc as bacc
import concourse.mybir as mybir
import concourse.tile as tile
from concourse.bass_utils import run_bass_kernel_spmd


P = 128
T = 2048          # target length
S = 2048          # source length
E = 1024          # embed dim
D = 64            # head dim
GC = 256          # channels per group (4 heads * 64)
NHL = 4           # heads per core (local)
KB = E // P       # 8 full k-blocks for the E contraction
TJ = 512          # t-chunk width
NTJ = T // TJ     # 4
NSB = S // P      # 16 s-blocks
VC = NHL * (D + 1)  # 260 = V-projection cols (64 V + 1 ones per head)
SCALE = float(D) ** -0.5  # 0.125
NEG = -1.0e30

F32 = mybir.dt.float32
BF16 = mybir.dt.bfloat16
F16 = mybir.dt.float16

# dtype knobs: fp16 runs the PE at 1 cyc/row (vs fp32's 2 half-speed passes)
# with an 11-bit mantissa; all tensors here are O(1)-scaled so range is safe.
DT_PROJ = F16     # Q/K/V projection matmul operand dtype (x tiles + weights)
DT_QK = F16       # QT/KT storage dtype -> scores matmul dtype
DT_EXP = F16      # exp output / V storage dtype -> attn@V matmul dtype
DT_AO = F16       # normalized attention-out dtype -> o-proj matmul dtype
NEG_H = -60000.0  # causal-mask additive constant (fits fp16; exp(scale*x)->0)


def _build_program():
    nc = bacc.Bacc()

    xq = nc.dram_tensor("xq_t", [E, T], DT_PROJ, kind="ExternalInput")
    xk = nc.dram_tensor("xk_t", [E, S], DT_PROJ, kind="ExternalInput")
    xv = nc.dram_tensor("xv_t", [E, S], DT_PROJ, kind="ExternalInput")
    wq = nc.dram_tensor("wq_t", [E, GC], DT_PROJ, kind="ExternalInput")
    wk = nc.dram_tensor("wk_t", [E, GC], DT_PROJ, kind="ExternalInput")
    wv = nc.dram_tensor("wv_t", [E + 1, VC], DT_PROJ, kind="ExternalInput")
    wo = nc.dram_tensor("wo_t", [GC, E], DT_AO, kind="ExternalInput")
    # constants: fp16 upper-triangular keep-mask [s, t] = (t >= s), and an f32
    # tensor holding key-padding bias columns plus per-channel q/k biases
    tri = nc.dram_tensor("tri", [P, P], DT_EXP, kind="ExternalInput")
    padb = nc.dram_tensor("padb", [P, NSB + 4], F32, kind="ExternalInput")
    out_t = nc.dram_tensor("out_t", [E, T], F32, kind="ExternalOutput")

    with tile.TileContext(nc) as tc:
        with (
            tc.tile_pool(name="consts", bufs=1) as cpool,
            tc.tile_pool(name="xs", bufs=10) as xpool,
            tc.tile_pool(name="persist", bufs=1) as ppool,
            tc.tile_pool(name="expw", bufs=6) as epool,
            tc.tile_pool(name="ao", bufs=1) as apool,
            tc.tile_pool(name="aon", bufs=2) as npool,
            tc.tile_pool(name="ft", bufs=2) as fpool,
            tc.tile_pool(name="ps", bufs=1, space="PSUM") as pspool,
        ):
            # ---- constants / weights to SBUF ----
            wq_sb = cpool.tile([P, KB, GC], DT_PROJ, name="wq_sb")
            wk_sb = cpool.tile([P, KB, GC], DT_PROJ, name="wk_sb")
            wv_sb = cpool.tile([P, KB + 1, VC], DT_PROJ, name="wv_sb")
            wo_sb = cpool.tile([P, 2, E], DT_AO, name="wo_sb")
            tri_sb = cpool.tile([P, P], DT_EXP, name="tri_sb")
            padb_sb = cpool.tile([P, NSB + 4], F32, name="padb_sb")
            ones_sb = cpool.tile([1, TJ], DT_PROJ, name="ones_sb")
            ones32_sb = cpool.tile([1, D], F32, name="ones32_sb")

            for w_sb, w_dram in ((wq_sb, wq), (wk_sb, wk)):
                nc.sync.dma_start(
                    w_sb[:], w_dram.rearrange("(kb p) c -> p kb c", p=P)
                )
            nc.sync.dma_start(
                wv_sb[:, :KB, :],
                wv[: KB * P, :].rearrange("(kb p) c -> p kb c", p=P),
            )
            nc.sync.dma_start(wv_sb[0:1, KB, :], wv[KB * P : KB * P + 1, :])
            nc.sync.dma_start(
                wo_sb[:], wo.rearrange("(cc p) o -> p cc o", p=P)
            )
            nc.sync.dma_start(tri_sb[:], tri[:])
            nc.sync.dma_start(padb_sb[:], padb[:])
            nc.any.memset(ones_sb[:], 1.0)
            nc.any.memset(ones32_sb[:], 1.0)

            # ---- persistent activations ----
            qt_sb = ppool.tile([P, 2, T], DT_QK, name="qt_sb")
            kt_sb = ppool.tile([P, 2, S], DT_QK, name="kt_sb")
            v_sb = ppool.tile([P, NSB, VC], DT_EXP, name="v_sb")
            aoTn = ppool.tile([P, 2, T], DT_AO, name="aoTn")

            # ---- Q / K projections (channel-major output) ----
            for ti, (x_dram, w_sb, dst) in enumerate(
                ((xq, wq_sb, qt_sb), (xk, wk_sb, kt_sb))
            ):
                for j in range(NTJ):
                    xt = []
                    for kb in range(KB):
                        t_ = xpool.tile([P, TJ], DT_PROJ, tag="xs", name="xt")
                        nc.sync.dma_start(
                            t_[:],
                            x_dram[kb * P : (kb + 1) * P, j * TJ : (j + 1) * TJ],
                        )
                        xt.append(t_)
                    for mc in range(2):
                        ps = pspool.tile([P, TJ], F32, tag="ps_pr", name="ps_pr")
                        for kb in range(KB):
                            nc.tensor.matmul(
                                ps[:],
                                lhsT=w_sb[:, kb, mc * P : (mc + 1) * P],
                                rhs=xt[kb][:],
                                start=(kb == 0),
                                stop=(kb == KB - 1),
                            )
                        nc.vector.tensor_scalar_add(
                            dst[:, mc, j * TJ : (j + 1) * TJ],
                            ps[:],
                            padb_sb[:, NSB + 2 * ti + mc : NSB + 2 * ti + mc + 1],
                        )

            # ---- V projection (sequence-major, 65 cols per head) ----
            for sj in range(NTJ):
                xt = []
                for kb in range(KB):
                    t_ = xpool.tile([P, TJ], DT_PROJ, tag="xs", name="xvt")
                    nc.sync.dma_start(
                        t_[:], xv[kb * P : (kb + 1) * P, sj * TJ : (sj + 1) * TJ]
                    )
                    xt.append(t_)
                for ii in range(TJ // P):
                    i = sj * (TJ // P) + ii
                    ps = pspool.tile([P, TJ], F32, tag="ps_pr", name="ps_v")
                    for kb in range(KB):
                        nc.tensor.matmul(
                            ps[:, :VC],
                            lhsT=xt[kb][:, ii * P : (ii + 1) * P],
                            rhs=wv_sb[:, kb, :],
                            start=(kb == 0),
                            stop=False,
                        )
                    nc.tensor.matmul(
                        ps[:, :VC],
                        lhsT=ones_sb[0:1, 0:P],
                        rhs=wv_sb[0:1, KB, :],
                        start=False,
                        stop=True,
                    )
                    nc.any.tensor_copy(out=v_sb[:, i, :], in_=ps[:, :VC])

            # ---- attention, head pairs (lh=0 at partitions 0-63, lh=1 at 64-127)
            for hp in range(2):
                aoTS = [
                    apool.tile([D + 1, T], F32, tag=f"aoTS_{lh}", name="aoTS")
                    for lh in range(2)
                ]
                for j in range(NTJ):
                    nsb_j = 4 * j + 4  # s-blocks 0..4j+3 (causal)
                    av_ps = [
                        pspool.tile([P, TJ], F32, tag=f"ps_av{lh}", name="ps_av")
                        for lh in range(2)
                    ]
                    ets = {}

                    def emit_scores(i, lh):
                        r = i - 4 * j
                        base = D * lh
                        ps = pspool.tile([P, TJ], F32, tag="ps_sc", name="ps_sc")
                        nc.tensor.matmul(
                            ps[:],
                            lhsT=kt_sb[base : base + D, hp, i * P : (i + 1) * P],
                            rhs=qt_sb[base : base + D, hp, j * TJ : (j + 1) * TJ],
                            start=True,
                            stop=True,
                        )
                        et = epool.tile([P, TJ], DT_EXP, tag="exp", name="et")
                        nc.scalar.activation(
                            et[:],
                            ps[:],
                            mybir.ActivationFunctionType.Exp,
                            bias=padb_sb[:, i : i + 1],
                            scale=SCALE,
                        )
                        if r >= 0:
                            # causal mask on the fp16 exp tile (DVE):
                            # cols < 128r are fully masked; the r-th 128-col
                            # sub-block is the triangular boundary
                            if r > 0:
                                nc.vector.memset(et[:, : P * r], 0.0)
                            nc.vector.tensor_mul(
                                out=et[:, P * r : P * (r + 1)],
                                in0=et[:, P * r : P * (r + 1)],
                                in1=tri_sb[:],
                            )
                        ets[(i, lh)] = et

                    def emit_av(i, lh):
                        h65 = (hp * 2 + lh) * (D + 1)
                        nc.tensor.matmul(
                            av_ps[lh][: D + 1, :],
                            lhsT=v_sb[:, i, h65 : h65 + D + 1],
                            rhs=ets.pop((i, lh)),
                            start=(i == 0),
                            stop=(i == nsb_j - 1),
                        )

                    # software pipeline: attn@V trails scores/exp by one
                    # s-block so the PE never stalls on the exp chain
                    for i in range(nsb_j):
                        for lh in range(2):
                            emit_scores(i, lh)
                        if i >= 1:
                            for lh in range(2):
                                emit_av(i - 1, lh)
                    for lh in range(2):
                        emit_av(nsb_j - 1, lh)
                    for lh in range(2):
                        nc.any.tensor_copy(
                            out=aoTS[lh][:, j * TJ : (j + 1) * TJ],
                            in_=av_ps[lh][: D + 1, :],
                        )
                # normalize the two heads of this pair
                for lh in range(2):
                    rrow = apool.tile([1, T], F32, tag="rrow", name="rrow")
                    nc.sync.dma_start(rrow[:], aoTS[lh][D : D + 1, :])
                    if lh == 0:
                        dst_slice = aoTn[0:D, hp, :]
                        tmp = None
                    else:
                        tmp = npool.tile([D, T], DT_AO, tag="aon", name="aon")
                        dst_slice = tmp[:]
                    for jj in range(NTJ):
                        rb = pspool.tile([P, TJ], F32, tag="ps_sc", name="ps_rb")
                        nc.tensor.matmul(
                            rb[:D, :],
                            lhsT=ones32_sb[0:1, 0:D],
                            rhs=rrow[0:1, jj * TJ : (jj + 1) * TJ],
                            start=True,
                            stop=True,
                        )
                        rbc = npool.tile([D, TJ], F32, tag="rbc", name="rbc", bufs=3)
                        nc.vector.reciprocal_approx_fast(rbc[:], rb[:D, :])
                        nc.vector.tensor_mul(
                            out=dst_slice[:, jj * TJ : (jj + 1) * TJ],
                            in0=aoTS[lh][0:D, jj * TJ : (jj + 1) * TJ],
                            in1=rbc[:],
                        )
                    if lh == 1:
                        # partition shift 0-63 -> 64-127 via SBUF-to-SBUF DMA
                        nc.sync.dma_start(aoTn[D : 2 * D, hp, :], tmp[:])

            # ---- output projection (partial, feature-major) ----
            for mc in range(KB):
                ft = fpool.tile([P, T], F32, tag="ft", name="ft")
                for j in range(NTJ):
                    ps = pspool.tile([P, TJ], F32, tag="ps_pr", name="ps_o")
                    for cc in range(2):
                        nc.tensor.matmul(
                            ps[:],
                            lhsT=wo_sb[:, cc, mc * P : (mc + 1) * P],
                            rhs=aoTn[:, cc, j * TJ : (j + 1) * TJ],
                            start=(cc == 0),
                            stop=(cc == 1),
                        )
                    nc.any.tensor_copy(out=ft[:, j * TJ : (j + 1) * TJ], in_=ps[:])
                nc.sync.dma_start(out_t[mc * P : (mc + 1) * P, :], ft[:])

    nc.compile()
    return nc


_NC_CACHE = None


def _get_nc():
    global _NC_CACHE
    if _NC_CACHE is None:
        _NC_CACHE = _build_program()
    return _NC_CACHE


def _make_in_maps(query, key, value, key_padding_mask, Wq, bq, Wk, bk, Wv, bv, Wo, bo):
    f32 = np.float32
    query = np.asarray(query, f32)
    key = np.asarray(key, f32)
    value = np.asarray(value, f32)
    kpm = np.asarray(key_padding_mask, bool)
    Wq, bq = np.asarray(Wq, f32), np.asarray(bq, f32)
    Wk, bk = np.asarray(Wk, f32), np.asarray(bk, f32)
    Wv, bv = np.asarray(Wv, f32), np.asarray(bv, f32)
    Wo = np.asarray(Wo, f32)

    # constants shared by all cores
    f16 = np.float16
    tri_np = (np.arange(P)[None, :] >= np.arange(P)[:, None]).astype(f16)

    in_maps = []
    for c in range(8):
        b, g = divmod(c, 4)
        cols = slice(g * GC, (g + 1) * GC)

        wq_t = Wq[cols, :].T.astype(f16)
        wk_t = Wk[cols, :].T.astype(f16)

        wv_t = np.zeros((E + 1, VC), f16)
        for h in range(NHL):
            ch = slice(g * GC + h * D, g * GC + (h + 1) * D)
            wv_t[:E, h * (D + 1) : h * (D + 1) + D] = Wv[ch, :].T
            wv_t[E, h * (D + 1) : h * (D + 1) + D] = bv[ch]
            wv_t[E, h * (D + 1) + D] = 1.0  # ones column -> softmax denominator

        wo_t = np.ascontiguousarray(Wo[:, cols].T.astype(f16))

        padb_np = np.where(kpm[b], NEG, 0.0).astype(f32).reshape(NSB, P).T
        biases = np.stack(
            [bq[cols][:P], bq[cols][P:], bk[cols][:P], bk[cols][P:]], axis=1
        ).astype(f32)
        padb_np = np.ascontiguousarray(np.concatenate([padb_np, biases], axis=1))

        in_maps.append(
            {
                "xq_t": np.ascontiguousarray(query[b].T.astype(f16)),
                "xk_t": np.ascontiguousarray(key[b].T.astype(f16)),
                "xv_t": np.ascontiguousarray(value[b].T.astype(f16)),
                "wq_t": np.ascontiguousarray(wq_t),
                "wk_t": np.ascontiguousarray(wk_t),
                "wv_t": wv_t,
                "wo_t": wo_t,
                "tri": tri_np,
                "padb": padb_np,
            }
        )
    return in_maps


def kernel(**inputs) -> np.ndarray:
    nc = _get_nc()
    in_maps = _make_in_maps(**inputs)
    res = run_bass_kernel_spmd(nc, in_maps, core_ids=list(range(8)))
    bo = np.asarray(inputs["bo"], np.float32)
    B = inputs["query"].shape[0]
    out = np.zeros((B, T, E), np.float32)
    for c in range(8):
        b = c // 4
        out[b] += res.results[c]["out_t"].T
    out += bo[None, None, :]
    return out


# revision 17
# speedup vs baseline: 3.6654x; 1.6614x over previous
"""Trainium2 Bass kernel for CrossAttention (B=2, T=S=2048, E=1024, H=16, D=64).

Sharding: 8 cores = 2 (batch) x 4 (head groups of 4 heads).
Each core computes, for its (b, g):
  - Q/K projections in feature-major layout: QT/KT = [256, 2048]
  - V projection in sequence-major layout with an appended ones column per
    head (gives the softmax denominator for free from the attn@V matmul)
  - causal flash-style attention (additive -1e30 mask folded into PSUM via an
    extra matmul with an identity lhsT; exp on ScalarE with the 1/sqrt(d)
    scale and per-partition key-padding bias folded in)
  - output projection partial: outT_partial = Wo[:, group].T-style [1024, 2048]
Host: shards/transposes inputs, gathers partials, sums over the 4 groups per
batch and adds bo.

All device transposes are avoided by preparing feature-major inputs host-side.
"""

import ml_dtypes
import numpy as np

import concourse.bass as bass
import concourse.bacc as bacc
import concourse.mybir as mybir
import concourse.tile as tile
from concourse.bass_utils import run_bass_kernel_spmd


P = 128
T = 2048          # target length
S = 2048          # source length
E = 1024          # embed dim
D = 64            # head dim
GC = 256          # channels per group (4 heads * 64)
NHL = 4           # heads per core (local)
KB = E // P       # 8 full k-blocks for the E contraction
TJ = 512          # t-chunk width
NTJ = T // TJ     # 4
NSB = S // P      # 16 s-blocks
VC = NHL * (D + 1)  # 260 = V-projection cols (64 V + 1 ones per head)
SCALE = float(D) ** -0.5  # 0.125
NEG = -1.0e30

F32 = mybir.dt.float32
BF16 = mybir.dt.bfloat16
F16 = mybir.dt.float16

# dtype knobs: fp16 runs the PE at 1 cyc/row (vs fp32's 2 half-speed passes)
# with an 11-bit mantissa; all tensors here are O(1)-scaled so range is safe.
DT_PROJ = F16     # Q/K/V projection matmul operand dtype (x tiles + weights)
DT_QK = F16       # QT/KT storage dtype -> scores matmul dtype
DT_EXP = F16      # exp output / V storage dtype -> attn@V matmul dtype
DT_AO = F16       # normalized attention-out dtype -> o-proj matmul dtype
NEG_H = -60000.0  # causal-mask additive constant (fits fp16; exp(scale*x)->0)


def _build_program():
    nc = bacc.Bacc()

    xq = nc.dram_tensor("xq_t", [E, T], DT_PROJ, kind="ExternalInput")
    xk = nc.dram_tensor("xk_t", [E, S], DT_PROJ, kind="ExternalInput")
    xv = nc.dram_tensor("xv_t", [E, S], DT_PROJ, kind="ExternalInput")
    wq = nc.dram_tensor("wq_t", [E, GC], DT_PROJ, kind="ExternalInput")
    wk = nc.dram_tensor("wk_t", [E, GC], DT_PROJ, kind="ExternalInput")
    wv = nc.dram_tensor("wv_t", [E + 1, VC], DT_PROJ, kind="ExternalInput")
    wo = nc.dram_tensor("wo_t", [GC, E], DT_AO, kind="ExternalInput")
    # constants: fp16 upper-triangular keep-mask [s, t] = (t >= s), and an f32
    # tensor holding key-padding bias columns plus per-channel q/k biases
    tri = nc.dram_tensor("tri", [P, 384 + P], DT_EXP, kind="ExternalInput")
    padb = nc.dram_tensor("padb", [P, NSB + 4], F32, kind="ExternalInput")
    out_t = nc.dram_tensor("out_t", [E, T], F32, kind="ExternalOutput")

    with tile.TileContext(nc) as tc:
        with (
            tc.tile_pool(name="consts", bufs=1) as cpool,
            tc.tile_pool(name="xs", bufs=10) as xpool,
            tc.tile_pool(name="persist", bufs=1) as ppool,
            tc.tile_pool(name="expw", bufs=4) as epool,
            tc.tile_pool(name="ao", bufs=1) as apool,
            tc.tile_pool(name="aon", bufs=2) as npool,
            tc.tile_pool(name="ft", bufs=2) as fpool,
            tc.tile_pool(name="ps", bufs=1, space="PSUM") as pspool,
        ):
            # ---- constants / weights to SBUF ----
            wq_sb = cpool.tile([P, KB, GC], DT_PROJ, name="wq_sb")
            wk_sb = cpool.tile([P, KB, GC], DT_PROJ, name="wk_sb")
            wv_sb = cpool.tile([P, KB + 1, VC], DT_PROJ, name="wv_sb")
            wo_sb = cpool.tile([P, 2, E], DT_AO, name="wo_sb")
            tri_sb = cpool.tile([P, 384 + P], DT_EXP, name="tri_sb")
            padb_sb = cpool.tile([P, NSB + 4], F32, name="padb_sb")
            ones_sb = cpool.tile([1, TJ], DT_PROJ, name="ones_sb")

            for w_sb, w_dram in ((wq_sb, wq), (wk_sb, wk)):
                nc.sync.dma_start(
                    w_sb[:], w_dram.rearrange("(kb p) c -> p kb c", p=P)
                )
            nc.sync.dma_start(
                wv_sb[:, :KB, :],
                wv[: KB * P, :].rearrange("(kb p) c -> p kb c", p=P),
            )
            nc.sync.dma_start(wv_sb[0:1, KB, :], wv[KB * P : KB * P + 1, :])
            nc.sync.dma_start(
                wo_sb[:], wo.rearrange("(cc p) o -> p cc o", p=P)
            )
            nc.sync.dma_start(tri_sb[:], tri[:])
            nc.sync.dma_start(padb_sb[:], padb[:])
            nc.any.memset(ones_sb[:], 1.0)

            # ---- persistent activations ----
            qt_sb = ppool.tile([P, 2, T], DT_QK, name="qt_sb")
            kt_sb = ppool.tile([P, 2, S], DT_QK, name="kt_sb")
            v_sb = ppool.tile([P, NSB, VC], DT_EXP, name="v_sb")
            aoTn = ppool.tile([P, 2, T], DT_AO, name="aoTn")

            # ---- Q / K projections (channel-major output) ----
            for ti, (x_dram, w_sb, dst) in enumerate(
                ((xq, wq_sb, qt_sb), (xk, wk_sb, kt_sb))
            ):
                for j in range(NTJ):
                    xt = []
                    for kb in range(KB):
                        t_ = xpool.tile([P, TJ], DT_PROJ, tag="xs", name="xt")
                        nc.sync.dma_start(
                            t_[:],
                            x_dram[kb * P : (kb + 1) * P, j * TJ : (j + 1) * TJ],
                        )
                        xt.append(t_)
                    for mc in range(2):
                        ps = pspool.tile([P, TJ], F32, tag="ps_pr", name="ps_pr", bufs=2)
                        for kb in range(KB):
                            nc.tensor.matmul(
                                ps[:],
                                lhsT=w_sb[:, kb, mc * P : (mc + 1) * P],
                                rhs=xt[kb][:],
                                start=(kb == 0),
                                stop=(kb == KB - 1),
                            )
                        nc.vector.tensor_scalar_add(
                            dst[:, mc, j * TJ : (j + 1) * TJ],
                            ps[:],
                            padb_sb[:, NSB + 2 * ti + mc : NSB + 2 * ti + mc + 1],
                        )

            # ---- V projection (sequence-major, 65 cols per head) ----
            for sj in range(NTJ):
                xt = []
                for kb in range(KB):
                    t_ = xpool.tile([P, TJ], DT_PROJ, tag="xs", name="xvt")
                    nc.sync.dma_start(
                        t_[:], xv[kb * P : (kb + 1) * P, sj * TJ : (sj + 1) * TJ]
                    )
                    xt.append(t_)
                for ii in range(TJ // P):
                    i = sj * (TJ // P) + ii
                    ps = pspool.tile([P, TJ], F32, tag="ps_pr", name="ps_v", bufs=2)
                    for kb in range(KB):
                        nc.tensor.matmul(
                            ps[:, :VC],
                            lhsT=xt[kb][:, ii * P : (ii + 1) * P],
                            rhs=wv_sb[:, kb, :],
                            start=(kb == 0),
                            stop=False,
                        )
                    nc.tensor.matmul(
                        ps[:, :VC],
                        lhsT=ones_sb[0:1, 0:P],
                        rhs=wv_sb[0:1, KB, :],
                        start=False,
                        stop=True,
                    )
                    nc.vector.tensor_scalar_mul(
                        v_sb[:, i, :], ps[:, :VC], padb_sb[:, i : i + 1]
                    )

            # ---- attention, head pairs (lh=0 at partitions 0-63, lh=1 at 64-127)
            for hp in range(2):
                aoTS = [
                    apool.tile([D + 1, T], F32, tag=f"aoTS_{lh}", name="aoTS")
                    for lh in range(2)
                ]
                for j in range(NTJ):
                    nsb_j = 4 * j + 4  # s-blocks 0..4j+3 (causal)
                    av_ps = [
                        pspool.tile([P, TJ], F32, tag=f"ps_av{lh}", name="ps_av", bufs=1)
                        for lh in range(2)
                    ]
                    ets = {}

                    def emit_scores_pair(m, lh):
                        # two s-blocks (2m, 2m+1) into one 2-bank psum tile,
                        # one exp over both halves (halves ACT inst + syncs)
                        base = D * lh
                        ps2 = pspool.tile([P, 2, TJ], F32, tag="ps_sc", name="ps_sc", bufs=2)
                        for u in range(2):
                            i = 2 * m + u
                            nc.tensor.matmul(
                                ps2[:, u, :],
                                lhsT=kt_sb[base : base + D, hp, i * P : (i + 1) * P],
                                rhs=qt_sb[base : base + D, hp, j * TJ : (j + 1) * TJ],
                                start=True,
                                stop=True,
                            )
                        et2 = epool.tile([P, 2, TJ], DT_EXP, tag="exp", name="et2")
                        nc.scalar.activation(
                            et2[:],
                            ps2[:],
                            mybir.ActivationFunctionType.Exp,
                            scale=SCALE,
                        )
                        for u in range(2):
                            r = 2 * m + u - 4 * j
                            if r >= 0:
                                # single fused causal mask: zeros then the
                                # triangular boundary, sliced from bigtri
                                w = P * (r + 1)
                                nc.vector.tensor_mul(
                                    out=et2[:, u, :w],
                                    in0=et2[:, u, :w],
                                    in1=tri_sb[:, 384 - P * r : 384 - P * r + w],
                                )
                        ets[(m, lh)] = et2

                    def emit_av_pair(m, lh):
                        h65 = (hp * 2 + lh) * (D + 1)
                        et2 = ets.pop((m, lh))
                        for u in range(2):
                            i = 2 * m + u
                            nc.tensor.matmul(
                                av_ps[lh][: D + 1, :],
                                lhsT=v_sb[:, i, h65 : h65 + D + 1],
                                rhs=et2[:, u, :],
                                start=(i == 0),
                                stop=(i == nsb_j - 1),
                            )

                    # software pipeline: attn@V trails scores/exp by one pair
                    npairs = nsb_j // 2
                    for m in range(npairs):
                        for lh in range(2):
                            emit_scores_pair(m, lh)
                        if m >= 1:
                            for lh in range(2):
                                emit_av_pair(m - 1, lh)
                    for lh in range(2):
                        emit_av_pair(npairs - 1, lh)
                    for lh in range(2):
                        nc.any.tensor_copy(
                            out=aoTS[lh][:, j * TJ : (j + 1) * TJ],
                            in_=av_ps[lh][: D + 1, :],
                        )
                # normalize the two heads of this pair
                for lh in range(2):
                    rrow = apool.tile([1, T], F32, tag="rrow", name="rrow")
                    nc.sync.dma_start(rrow[:], aoTS[lh][D : D + 1, :])
                    rcp = apool.tile([1, T], F32, tag="rcp", name="rcp")
                    nc.vector.reciprocal_approx_fast(rcp[:], rrow[:])
                    # replicate the reciprocal row across 64 partitions (GpSimd)
                    rb64 = npool.tile([D, T], F32, tag="rb64", name="rb64")
                    nc.gpsimd.partition_broadcast(rb64[:], rcp[0:1, :])
                    if lh == 0:
                        dst_slice = aoTn[0:D, hp, :]
                        tmp = None
                    else:
                        tmp = npool.tile([D, T], DT_AO, tag="aon", name="aon")
                        dst_slice = tmp[:]
                    nc.vector.tensor_mul(
                        out=dst_slice[:], in0=aoTS[lh][0:D, :], in1=rb64[:]
                    )
                    if lh == 1:
                        # partition shift 0-63 -> 64-127 via SBUF-to-SBUF DMA
                        nc.sync.dma_start(aoTn[D : 2 * D, hp, :], tmp[:])

            # ---- output projection (partial, feature-major) ----
            for mc in range(KB):
                ft = fpool.tile([P, T], F32, tag="ft", name="ft")
                for j in range(NTJ):
                    ps = pspool.tile([P, TJ], F32, tag="ps_pr", name="ps_o", bufs=2)
                    for cc in range(2):
                        nc.tensor.matmul(
                            ps[:],
                            lhsT=wo_sb[:, cc, mc * P : (mc + 1) * P],
                            rhs=aoTn[:, cc, j * TJ : (j + 1) * TJ],
                            start=(cc == 0),
                            stop=(cc == 1),
                        )
                    nc.any.tensor_copy(out=ft[:, j * TJ : (j + 1) * TJ], in_=ps[:])
                nc.sync.dma_start(out_t[mc * P : (mc + 1) * P, :], ft[:])

    nc.compile()
    return nc


_NC_CACHE = None


def _get_nc():
    global _NC_CACHE
    if _NC_CACHE is None:
        _NC_CACHE = _build_program()
    return _NC_CACHE


def _make_in_maps(query, key, value, key_padding_mask, Wq, bq, Wk, bk, Wv, bv, Wo, bo):
    f32 = np.float32
    query = np.asarray(query, f32)
    key = np.asarray(key, f32)
    value = np.asarray(value, f32)
    kpm = np.asarray(key_padding_mask, bool)
    Wq, bq = np.asarray(Wq, f32), np.asarray(bq, f32)
    Wk, bk = np.asarray(Wk, f32), np.asarray(bk, f32)
    Wv, bv = np.asarray(Wv, f32), np.asarray(bv, f32)
    Wo = np.asarray(Wo, f32)

    # constants shared by all cores
    f16 = np.float16
    tri_small = (np.arange(P)[None, :] >= np.arange(P)[:, None]).astype(f16)
    tri_np = np.concatenate([np.zeros((P, 384), f16), tri_small], axis=1)

    in_maps = []
    for c in range(8):
        b, g = divmod(c, 4)
        cols = slice(g * GC, (g + 1) * GC)

        wq_t = Wq[cols, :].T.astype(f16)
        wk_t = Wk[cols, :].T.astype(f16)

        wv_t = np.zeros((E + 1, VC), f16)
        for h in range(NHL):
            ch = slice(g * GC + h * D, g * GC + (h + 1) * D)
            wv_t[:E, h * (D + 1) : h * (D + 1) + D] = Wv[ch, :].T
            wv_t[E, h * (D + 1) : h * (D + 1) + D] = bv[ch]
            wv_t[E, h * (D + 1) + D] = 1.0  # ones column -> softmax denominator

        wo_t = np.ascontiguousarray(Wo[:, cols].T.astype(f16))

        padb_np = np.where(kpm[b], 0.0, 1.0).astype(f32).reshape(NSB, P).T
        biases = np.stack(
            [bq[cols][:P], bq[cols][P:], bk[cols][:P], bk[cols][P:]], axis=1
        ).astype(f32)
        padb_np = np.ascontiguousarray(np.concatenate([padb_np, biases], axis=1))

        in_maps.append(
            {
                "xq_t": np.ascontiguousarray(query[b].T.astype(f16)),
                "xk_t": np.ascontiguousarray(key[b].T.astype(f16)),
                "xv_t": np.ascontiguousarray(value[b].T.astype(f16)),
                "wq_t": np.ascontiguousarray(wq_t),
                "wk_t": np.ascontiguousarray(wk_t),
                "wv_t": wv_t,
                "wo_t": wo_t,
                "tri": tri_np,
                "padb": padb_np,
            }
        )
    return in_maps


def kernel(**inputs) -> np.ndarray:
    nc = _get_nc()
    in_maps = _make_in_maps(**inputs)
    res = run_bass_kernel_spmd(nc, in_maps, core_ids=list(range(8)))
    bo = np.asarray(inputs["bo"], np.float32)
    B = inputs["query"].shape[0]
    out = np.zeros((B, T, E), np.float32)
    for c in range(8):
        b = c // 4
        out[b] += res.results[c]["out_t"].T
    out += bo[None, None, :]
    return out
